# revision 18
# baseline (speedup 1.0000x reference)
"""DeformableConv2d Trainium2 kernel.

Shapes (hardcoded): x[4,64,128,128], offset/modulator maps [4,64,128,128],
offset_w[18,64,3,3], offset_b[18], mod_w[9,64,3,3], mod_b[9], weight[64,64,3,3].
Output [4,64,128,128] f32.

Sharding: 8 cores; core c -> batch b=c//2, row-half hh=c%2 (output rows
h0=64*hh .. h0+64). Each core keeps the full image of its batch for the
(clipped) bilinear gathers.

Algorithm per core:
  A) Build padded NHWC "pair" image xpd[r, cw, 2, 64] in DRAM from x (PE
     transposes). Record at (r,cw) = pixels (r-1,cw-1),(r-1,cw),(r,cw-1),(r,cw)
     of the image with 1-px zero border; one 1KB dma_gather record = all 4
     bilinear corners for all 64 channels.
  B) offset/mod convs as 9-tap accumulated matmuls (bf16 in, f32 psum),
     PE-transpose to pixel-on-partition layout, add host-baked base+bias grid.
  C) Exact floor/clip/validity/bilinear-weight math on DVE (f32), int16
     gather indices staged via DRAM into the 16-partition wrapped layout.
  D) dma_gather 2x2x64 records; scale by per-corner weights (broadcast AP),
     combine corners, PE-transpose, accumulate out = sum_k W_k^T @ samp_k.
"""

import numpy as np

B, C, H, W = 4, 64, 128, 128
K = 3
K2 = 9
NCO = 8          # cores
HH = H // 2      # rows per core (64)
NPIX = HH * W    # 8192 pixels per core
NT = HH          # 64 pixel tiles of 128 (one image row each)
PW = W + 2       # 130 padded width
PR = H + 2       # 130 padded rows
REC = 2 * 2 * C  # 256 f32 per gather record
XPD_ROWS = (PR - 1) * PW + PW + 64   # generous slack beyond max idx reads
CH27 = 2 * K2 + K2                   # 18 offset + 9 mod channels
CHP = 48         # padded channel layout: 0:18 offs, 32:41 mod, rest junk
TCH = 4          # pixel tiles per matmul chunk (512 px)
NCHUNK = NT // TCH                   # 16 chunks
IDX_PER_CHUNK = TCH * K2 * 128       # 4608

_CACHE = {}


def _build_module(phase=4):
    import concourse.bass as bass
    import concourse.bacc as bacc
    import concourse.mybir as mybir
    from concourse.tile import TileContext
    from concourse.alu_op_type import AluOpType as alu
    from concourse.bass_types import AP
    from concourse import library_config

    f32 = mybir.dt.float32
    bf16 = mybir.dt.bfloat16
    i16 = mybir.dt.int16
    i32 = mybir.dt.int32
    ACT = mybir.ActivationFunctionType

    nc = bacc.Bacc()

    xin = nc.dram_tensor("xin", (C, H * W), f32, kind="ExternalInput")
    om = nc.dram_tensor("om", (C, 66 * PW), bf16, kind="ExternalInput")
    mm = nc.dram_tensor("mm", (C, 66 * PW), bf16, kind="ExternalInput")
    wmain = nc.dram_tensor("wmain", (C, K2 * 64), bf16, kind="ExternalInput")
    wconv = nc.dram_tensor("wconv", (C, K2 * CH27), bf16, kind="ExternalInput")
    cgrid = nc.dram_tensor("cgrid", (128, NT * CHP), f32, kind="ExternalInput")
    ident = nc.dram_tensor("ident", (128, 128), f32, kind="ExternalInput")
    out = nc.dram_tensor("out", (C, NPIX), f32, kind="ExternalOutput")

    xpd = nc.dram_tensor("xpd", (XPD_ROWS * 128,), bf16, kind="Internal")
    didx = nc.dram_tensor("didx", (NT * K2 * 128,), i16, kind="Internal")

    with TileContext(nc) as tc:
        # ---------------- persistent pools ----------------
        with (
            tc.tile_pool(name="const", bufs=1) as constp,
            tc.tile_pool(name="offall", bufs=1) as offp,
            tc.tile_pool(name="scal", bufs=1) as scalp,
            tc.tile_pool(name="wq", bufs=1) as wqp,
            tc.tile_pool(name="outsb", bufs=1) as outp,
        ):
            idn = constp.tile([128, 128], f32)
            nc.sync.dma_start(idn[:, :], ident[:, :])
            idn_b = constp.tile([128, 128], bf16)
            nc.vector.tensor_copy(idn_b[:, :], idn[:, :])
            wmain_sb = constp.tile([C, K2 * 64], bf16)
            nc.sync.dma_start(wmain_sb[:, :], wmain[:, :])
            wconv_sb = constp.tile([C, K2 * CH27], bf16)
            nc.sync.dma_start(wconv_sb[:, :], wconv[:, :])
            cg = constp.tile([128, NT * CHP], f32)
            nc.sync.dma_start(cg[:, :], cgrid[:, :])

            off_all = offp.tile([128, NT * CHP], f32)  # [p, t, ch32]
            out_sb = outp.tile([C, NPIX], f32)

            # ---------- Phase A: build xpd ----------
            with (
                tc.tile_pool(name="xs", bufs=1) as xsp,
                tc.tile_pool(name="nh", bufs=1) as nhp,
                tc.tile_pool(name="pst", bufs=2, space="PSUM") as pstp,
                tc.tile_pool(name="zs", bufs=1) as zsp,
            ):
                xs = xsp.tile([C, H * W], f32)
                nc.sync.dma_start(xs[:, :], xin[:, :])
                nh = nhp.tile([128, H * C], bf16)  # [w, h*64+c]
                for g in range(16):  # groups of 8 rows
                    ps = pstp.tile([128, 512], f32)
                    for j in range(8):
                        h = g * 8 + j
                        nc.tensor.transpose(
                            ps[:, j * 64:(j + 1) * 64],
                            xs[:, h * W:(h + 1) * W],
                            idn[:C, :C],
                        )
                    nc.scalar.copy(nh[:, g * 512:(g + 1) * 512], ps[:, :])

                # zero border strips of xpd
                zs = zsp.tile([128, 130], bf16)
                nc.vector.memset(zs[:, :], 0.0)
                # row r=0 both slots (slot1 rewritten by interior later)
                nc.sync.dma_start(
                    xpd[0:130 * 128].rearrange("(p x) -> p x", x=130), zs[:, :]
                )
                # row r=128 both slots (slot0 rewritten by interior later)
                nc.sync.dma_start(
                    xpd[128 * PW * 128:128 * PW * 128 + 130 * 128]
                    .rearrange("(p x) -> p x", x=130),
                    zs[:, :],
                )
                xv = xpd[0:PR * PW * 128].rearrange("(r y x) -> r y x", y=PW, x=128)
                nc.sync.dma_start(xv[0:65, 0, :], zs[0:65, 0:128])
                nc.sync.dma_start(xv[65:130, 0, :], zs[0:65, 0:128])
                nc.sync.dma_start(xv[0:65, PW - 1, :], zs[0:65, 0:128])
                nc.sync.dma_start(xv[65:130, PW - 1, :], zs[0:65, 0:128])
                # interior: slot0 at record-row h+1, slot1 at record-row h
                nh_v = nh[:, :].rearrange("p (h c) -> p h c", c=64)
                xpd_v = xpd[0:PR * PW * 128].rearrange(
                    "(r y s c) -> r y s c", y=PW, s=2, c=64)
                nc.sync.dma_start(
                    xpd_v[1:129, 1:129, 0, :].transpose([1, 0, 2]), nh_v
                )
                nc.sync.dma_start(
                    xpd_v[0:128, 1:129, 1, :].transpose([1, 0, 2]), nh_v
                )

            # ---------- Phase B: convs ----------
            with (
                tc.tile_pool(name="oms", bufs=1) as omsp,
                tc.tile_pool(name="cps", bufs=2, space="PSUM") as cpsp,
                tc.tile_pool(name="cos", bufs=2) as cosp,
                tc.tile_pool(name="pst2", bufs=2, space="PSUM") as pst2p,
            ):
                oms = omsp.tile([C, 66 * PW], bf16)
                nc.sync.dma_start(oms[:, :], om[:, :])
                mms = omsp.tile([C, 66 * PW], bf16)
                nc.sync.dma_start(mms[:, :], mm[:, :])
                oms_v = oms[:, :].rearrange("c (r y) -> c r y", y=PW)
                mms_v = mms[:, :].rearrange("c (r y) -> c r y", y=PW)
                wconv_v = wconv_sb[:, :].rearrange("c (t j) -> c t j", j=CH27)

                for ch in range(NCHUNK):  # 4 output rows per chunk
                    r0 = ch * TCH
                    ps_o = cpsp.tile([2 * K2, 512], f32, tag="pso")
                    ps_m = cpsp.tile([K2, 512], f32, tag="psm")
                    for tap in range(K2):
                        ki, kj = tap // K, tap % K
                        rhs_o = oms_v[:, ki + r0: ki + r0 + TCH, kj: kj + W]
                        rhs_m = mms_v[:, ki + r0: ki + r0 + TCH, kj: kj + W]
                        nc.tensor.matmul(
                            ps_o[:, :], wconv_v[:, tap, 0:2 * K2], rhs_o,
                            start=(tap == 0), stop=(tap == K2 - 1),
                        )
                        nc.tensor.matmul(
                            ps_m[:, :], wconv_v[:, tap, 2 * K2:CH27], rhs_m,
                            start=(tap == 0), stop=(tap == K2 - 1),
                        )
                    co = cosp.tile([CHP, 512], f32, tag="co")
                    nc.vector.memset(co[:, :], 0.0)
                    nc.scalar.copy(co[0:2 * K2, :], ps_o[:, :])
                    nc.scalar.copy(co[32:32 + K2, :], ps_m[:, :])
                    ps_t = pst2p.tile([128, TCH * CHP], f32, tag="pst2")
                    for j in range(TCH):
                        nc.tensor.transpose(
                            ps_t[:, j * CHP:(j + 1) * CHP],
                            co[:, j * 128:(j + 1) * 128],
                            idn[:CHP, :CHP],
                        )
                    nc.scalar.copy(
                        off_all[:, (r0) * CHP:(r0 + TCH) * CHP], ps_t[:, :]
                    )

            # add base+bias grid -> off_all holds py, px, mc directly
            nc.vector.tensor_tensor(off_all[:, :], off_all[:, :], cg[:, :], alu.add)

            # ---------- Phase C: index & weight math ----------
            oav = off_all[:, :].rearrange("p (t j) -> p t j", j=CHP)
            py = oav[:, :, 0:2 * K2:2]
            px = oav[:, :, 1:2 * K2:2]
            mc = oav[:, :, 32:32 + K2]

            NK = NT * K2  # 576
            s_i32 = scalp.tile([128, NK], i32)
            y0 = scalp.tile([128, NK], f32)
            x0 = scalp.tile([128, NK], f32)
            tmp = scalp.tile([128, NK], f32)
            tmp2 = scalp.tile([128, NK], f32)
            wy = scalp.tile([128, NK], f32)
            wx = scalp.tile([128, NK], f32)
            ay0 = scalp.tile([128, NK], f32)
            ay1 = scalp.tile([128, NK], f32)
            bx0 = scalp.tile([128, NK], f32)
            bx1 = scalp.tile([128, NK], f32)
            msk2 = scalp.tile([128, NK], f32)
            idxf = scalp.tile([128, NK], f32)
            idx16 = scalp.tile([128, NK], i16)
            wq = wqp.tile([128, NT * K2 * 4], f32)  # [p, t, k, 4]
            wq16 = wqp.tile([128, NT * K2 * 4], bf16)

            def floor_exact(dst, src):
                nc.vector.tensor_copy(s_i32[:, :], src)
                nc.vector.tensor_copy(dst, s_i32[:, :])
                nc.vector.tensor_tensor(tmp[:, :], dst, src, alu.is_gt)
                nc.vector.tensor_tensor(dst, dst, tmp[:, :], alu.subtract)

            floor_exact(y0[:, :], py)
            floor_exact(x0[:, :], px)
            nc.vector.tensor_tensor(wy[:, :], py, y0[:, :], alu.subtract)
            nc.vector.tensor_tensor(wx[:, :], px, x0[:, :], alu.subtract)

            # mask2 = 2*sigmoid(mc)
            nc.scalar.activation(msk2[:, :], mc, ACT.Sigmoid)
            nc.vector.tensor_scalar(msk2[:, :], msk2[:, :], 2.0, None, alu.mult)

            # validity
            def valid(dst, src, lo, hi):
                nc.vector.tensor_scalar(tmp[:, :], src, hi, None, alu.is_le)
                nc.vector.scalar_tensor_tensor(
                    dst, src, lo, tmp[:, :], alu.is_ge, alu.mult
                )

            valid(tmp2[:, :], y0[:, :], 0.0, 127.0)    # vy0
            # ay0 = (1-wy)*vy0*msk2
            nc.vector.tensor_scalar(ay0[:, :], wy[:, :], -1.0, 1.0, alu.mult, alu.add)
            nc.vector.tensor_tensor(ay0[:, :], ay0[:, :], tmp2[:, :], alu.mult)
            nc.vector.tensor_tensor(ay0[:, :], ay0[:, :], msk2[:, :], alu.mult)
            valid(tmp2[:, :], y0[:, :], -1.0, 126.0)   # vy1
            nc.vector.tensor_tensor(ay1[:, :], wy[:, :], tmp2[:, :], alu.mult)
            nc.vector.tensor_tensor(ay1[:, :], ay1[:, :], msk2[:, :], alu.mult)
            valid(tmp2[:, :], x0[:, :], 0.0, 127.0)    # vx0
            nc.vector.tensor_scalar(bx0[:, :], wx[:, :], -1.0, 1.0, alu.mult, alu.add)
            nc.vector.tensor_tensor(bx0[:, :], bx0[:, :], tmp2[:, :], alu.mult)
            valid(tmp2[:, :], x0[:, :], -1.0, 126.0)   # vx1
            nc.vector.tensor_tensor(bx1[:, :], wx[:, :], tmp2[:, :], alu.mult)

            wqv = wq[:, :].rearrange("p (t k c) -> p t k c", k=K2, c=4)
            # record order: [col0:(top,bot), col1:(top,bot)] = w00,w10,w01,w11
            nc.vector.tensor_tensor(wqv[:, :, :, 0], ay0[:, :].rearrange("p (t k) -> p t k", k=K2), bx0[:, :].rearrange("p (t k) -> p t k", k=K2), alu.mult)
            nc.vector.tensor_tensor(wqv[:, :, :, 1], ay1[:, :].rearrange("p (t k) -> p t k", k=K2), bx0[:, :].rearrange("p (t k) -> p t k", k=K2), alu.mult)
            nc.vector.tensor_tensor(wqv[:, :, :, 2], ay0[:, :].rearrange("p (t k) -> p t k", k=K2), bx1[:, :].rearrange("p (t k) -> p t k", k=K2), alu.mult)
            nc.vector.tensor_tensor(wqv[:, :, :, 3], ay1[:, :].rearrange("p (t k) -> p t k", k=K2), bx1[:, :].rearrange("p (t k) -> p t k", k=K2), alu.mult)

            # idx = (clip(y0,-1,127)+1)*130 + clip(x0,-1,127)+1
            nc.vector.tensor_scalar(tmp[:, :], y0[:, :], -1.0, 127.0, alu.max, alu.min)
            nc.vector.tensor_scalar(tmp2[:, :], x0[:, :], -1.0, 127.0, alu.max, alu.min)
            nc.vector.scalar_tensor_tensor(
                idxf[:, :], tmp[:, :], float(PW), tmp2[:, :], alu.mult, alu.add
            )
            nc.vector.tensor_scalar(idxf[:, :], idxf[:, :], float(PW + 1), None, alu.add)
            nc.vector.tensor_copy(idx16[:, :], idxf[:, :])

            # stage indices: didx[(t*9+k)*128 + p] = idx16[p, t, k]
            nc.sync.dma_start(
                didx[:].rearrange("(t k p) -> t k p", k=K2, p=128).transpose([2, 0, 1]),
                idx16[:, :].rearrange("p (t k) -> p t k", k=K2),
            )

            # ---------- Phase D/E ----------
            with (
                tc.tile_pool(name="idxa", bufs=1) as idxap,
                tc.tile_pool(name="rec", bufs=3) as recp,
                tc.tile_pool(name="samp", bufs=3) as sampp,
                tc.tile_pool(name="pst3", bufs=2, space="PSUM") as pst3p,
                tc.tile_pool(name="rhs", bufs=2) as rhsp,
                tc.tile_pool(name="ops", bufs=2, space="PSUM") as opsp,
            ):
                idxall = idxap.tile([128, NT * K2 * 128 // 16], i16)  # [128, 4608]
                didx_t = didx[:].rearrange("(s p) -> s p", p=16).transpose([1, 0])
                for r in range(8):
                    nc.sync.dma_start(idxall[r * 16:(r + 1) * 16, :], didx_t)

                nc.vector.tensor_copy(wq16[:, :], wq[:, :])
                wq16v = wq16[:, :].rearrange("p (t k c) -> p t k c", k=K2, c=4)
                wmv = wmain_sb[:, :].rearrange("c (k o) -> c k o", o=64)
                xpd_src = xpd[:].rearrange("(r e) -> r e", e=128)[0:XPD_ROWS - 64, :]
                xpd_src = xpd_src.rearrange("r e -> r e")  # [rows,128]; elem 256 via 2 rows

                for chk in range(NCHUNK):
                    t0 = chk * TCH
                    rec = recp.tile([128, TCH * K2 * REC], bf16, tag="rec")
                    xpd_gsrc = AP(
                        xpd[:].tensor, 0,
                        [[128, XPD_ROWS - 2], [1, REC]],
                    )
                    nc.gpsimd.dma_gather(
                        rec[:, :].rearrange("p (n e) -> p n e", e=REC),
                        xpd_gsrc,
                        idxall[:, chk * (IDX_PER_CHUNK // 16):(chk + 1) * (IDX_PER_CHUNK // 16)],
                        num_idxs=IDX_PER_CHUNK,
                        num_idxs_reg=IDX_PER_CHUNK,
                        elem_size=REC,
                        elem_step=128,
                        single_packet=False,
                    )
                    # scale by corner weights (in-place), combine corners
                    recv = rec[:, :].rearrange("p (n c e) -> p n c e", c=4, e=64)
                    wslice = wq16v[:, t0:t0 + TCH, :, :].rearrange("p t k c -> p (t k) c")
                    nc.vector.tensor_tensor(
                        recv, recv, wslice.to_broadcast((128, TCH * K2, 4, 64)), alu.mult
                    )
                    samp = sampp.tile([128, TCH * K2 * 64], bf16, tag="samp")
                    sampv = samp[:, :].rearrange("p (n e) -> p n e", e=64)
                    nc.vector.tensor_tensor(
                        sampv,
                        recv[:, :, 0, :], recv[:, :, 1, :], alu.add,
                    )
                    nc.vector.tensor_tensor(
                        sampv, sampv, recv[:, :, 2, :], alu.add,
                    )
                    nc.vector.tensor_tensor(
                        sampv, sampv, recv[:, :, 3, :], alu.add,
                    )
                    ops_t = opsp.tile([C, 512], f32, tag="ops")
                    for kg in range(3):
                        ps3 = pst3p.tile([C, 3 * 512], bf16, tag="ps3")
                        for k3 in range(3):
                            k = kg * 3 + k3
                            for j in range(TCH):
                                nc.tensor.transpose(
                                    ps3[:, k3 * 512 + j * 128:k3 * 512 + (j + 1) * 128],
                                    sampv[:, (j * K2 + k), :],
                                    idn_b[:, :],
                                )
                        rhs = rhsp.tile([C, 3 * 512], bf16, tag="rhs")
                        nc.scalar.copy(rhs[:, :], ps3[:, :])
                        for k3 in range(3):
                            k = kg * 3 + k3
                            nc.tensor.matmul(
                                ops_t[:, :], wmv[:, k, :],
                                rhs[:, k3 * 512:(k3 + 1) * 512],
                                start=(k == 0), stop=(k == K2 - 1),
                            )
                    nc.scalar.copy(out_sb[:, t0 * W:(t0 + TCH) * W], ops_t[:, :])

            nc.sync.dma_start(out[:, :], out_sb[:, :])

    nc.compile()
    return nc


def _prep_core_inputs(inputs, core):
    x = np.asarray(inputs["x"], np.float32)
    omap = np.asarray(inputs["offset_map"], np.float32)
    mmap = np.asarray(inputs["modulator_map"], np.float32)
    ow = np.asarray(inputs["offset_w"], np.float32)
    ob = np.asarray(inputs["offset_b"], np.float32)
    mw = np.asarray(inputs["mod_w"], np.float32)
    mb = np.asarray(inputs["mod_b"], np.float32)
    wt = np.asarray(inputs["weight"], np.float32)
    import ml_dtypes

    b, hh = core // 2, core % 2
    h0 = hh * HH

    def slab(a):
        s = np.zeros((C, 66, PW), np.float32)
        lo, hi = h0 - 1, h0 + HH + 1  # global rows [lo, hi)
        glo, ghi = max(lo, 0), min(hi, H)
        s[:, glo - lo: ghi - lo, 1:W + 1] = a[b, :, glo:ghi, :]
        return s.reshape(C, 66 * PW).astype(ml_dtypes.bfloat16)

    wmain = np.ascontiguousarray(
        wt.reshape(C, C, K2).transpose(1, 2, 0).reshape(C, K2 * 64)
    ).astype(ml_dtypes.bfloat16)
    catw = np.concatenate([ow.reshape(2 * K2, C, K2), mw.reshape(K2, C, K2)], 0)
    wconv = catw.transpose(1, 2, 0).reshape(C, K2 * CH27)
    wconv = np.ascontiguousarray(wconv).astype(ml_dtypes.bfloat16)

    cgrid = np.zeros((128, NT, CHP), np.float32)
    p = np.arange(128)
    t = np.arange(NT)
    for k in range(K2):
        ki, kj = k // K, k % K
        cgrid[:, :, 2 * k] = ob[2 * k] + (h0 + t[None, :]) + ki - 1
        cgrid[:, :, 2 * k + 1] = ob[2 * k + 1] + p[:, None] + kj - 1
        cgrid[:, :, 32 + k] = mb[k]
    ident = np.eye(128, dtype=np.float32)

    return {
        "xin": x[b].reshape(C, H * W).copy(),
        "om": slab(omap),
        "mm": slab(mmap),
        "wmain": wmain,
        "wconv": wconv,
        "cgrid": cgrid.reshape(128, NT * CHP).copy(),
        "ident": ident,
    }


def get_module():
    import os
    phase = int(os.environ.get("KPHASE", "4"))
    key = ("nc", phase)
    if key not in _CACHE:
        _CACHE[key] = _build_module(phase)
    return _CACHE[key]


def kernel(**inputs) -> np.ndarray:
    import os
    from concourse.bass_utils import run_bass_kernel_spmd

    nc = get_module()
    in_maps = [_prep_core_inputs(inputs, c) for c in range(NCO)]
    trace = bool(int(os.environ.get("KBENCH_TRACE", "0")))
    res = run_bass_kernel_spmd(nc, in_maps, core_ids=list(range(NCO)), trace=trace)
    _CACHE["last_results"] = res
    out = np.zeros((B, C, H, W), np.float32)
    for c in range(NCO):
        b, hh = c // 2, c % 2
        out[b, :, hh * HH:(hh + 1) * HH, :] = res.results[c]["out"].reshape(C, HH, W)
    return out


# revision 25
# speedup vs baseline: 1.2055x; 1.2055x over previous
"""DeformableConv2d Trainium2 kernel.

Shapes (hardcoded): x[4,64,128,128], offset/modulator maps [4,64,128,128],
offset_w[18,64,3,3], offset_b[18], mod_w[9,64,3,3], mod_b[9], weight[64,64,3,3].
Output [4,64,128,128] f32.

Sharding: 8 cores; core c -> batch b=c//2, row-half hh=c%2 (output rows
h0=64*hh .. h0+64). Each core keeps the full image of its batch for the
(clipped) bilinear gathers.

Algorithm per core:
  A) Build padded NHWC "pair" image xpd[r, cw, 2, 64] in DRAM from x (PE
     transposes). Record at (r,cw) = pixels (r-1,cw-1),(r-1,cw),(r,cw-1),(r,cw)
     of the image with 1-px zero border; one 1KB dma_gather record = all 4
     bilinear corners for all 64 channels.
  B) offset/mod convs as 9-tap accumulated matmuls (bf16 in, f32 psum),
     PE-transpose to pixel-on-partition layout, add host-baked base+bias grid.
  C) Exact floor/clip/validity/bilinear-weight math on DVE (f32), int16
     gather indices staged via DRAM into the 16-partition wrapped layout.
  D) dma_gather 2x2x64 records; scale by per-corner weights (broadcast AP),
     combine corners, PE-transpose, accumulate out = sum_k W_k^T @ samp_k.
"""

import numpy as np

B, C, H, W = 4, 64, 128, 128
K = 3
K2 = 9
NCO = 8          # cores
HH = H // 2      # rows per core (64)
NPIX = HH * W    # 8192 pixels per core
NT = HH          # 64 pixel tiles of 128 (one image row each)
PW = W + 2       # 130 padded width
PR = H + 2       # 130 padded rows
REC = 2 * 2 * C  # 256 f32 per gather record
XPD_ROWS = (PR - 1) * PW + PW + 64   # generous slack beyond max idx reads
CH27 = 2 * K2 + K2                   # 18 offset + 9 mod channels
CHP = 48         # padded channel layout: 0:18 offs, 32:41 mod, rest junk
TCH = 4          # pixel tiles per matmul chunk (512 px)
NCHUNK = NT // TCH                   # 16 chunks
IDX_PER_CHUNK = TCH * K2 * 128       # 4608

_CACHE = {}


def _build_module(phase=4):
    import concourse.bass as bass
    import concourse.bacc as bacc
    import concourse.mybir as mybir
    from concourse.tile import TileContext
    from concourse.alu_op_type import AluOpType as alu
    from concourse.bass_types import AP
    from concourse import library_config

    f32 = mybir.dt.float32
    bf16 = mybir.dt.bfloat16
    i16 = mybir.dt.int16
    i32 = mybir.dt.int32
    ACT = mybir.ActivationFunctionType

    nc = bacc.Bacc()

    xin = nc.dram_tensor("xin", (C, H * W), f32, kind="ExternalInput")
    om = nc.dram_tensor("om", (C, 66 * PW), bf16, kind="ExternalInput")
    mm = nc.dram_tensor("mm", (C, 66 * PW), bf16, kind="ExternalInput")
    wmain = nc.dram_tensor("wmain", (C, K2 * 64), bf16, kind="ExternalInput")
    wconv = nc.dram_tensor("wconv", (128, 6 * CH27), bf16, kind="ExternalInput")
    cgrid = nc.dram_tensor("cgrid", (128, NT * CHP), f32, kind="ExternalInput")
    ident = nc.dram_tensor("ident", (128, 128), f32, kind="ExternalInput")
    out = nc.dram_tensor("out", (C, NPIX), f32, kind="ExternalOutput")

    xpd = nc.dram_tensor("xpd", (XPD_ROWS * 128,), bf16, kind="Internal")
    didx = nc.dram_tensor("didx", (NT * K2 * 128,), i16, kind="Internal")

    with TileContext(nc) as tc:
        # ---------------- persistent pools ----------------
        with (
            tc.tile_pool(name="const", bufs=1) as constp,
            tc.tile_pool(name="offall", bufs=1) as offp,
            tc.tile_pool(name="scal", bufs=1) as scalp,
            tc.tile_pool(name="wq", bufs=1) as wqp,
            tc.tile_pool(name="outsb", bufs=1) as outp,
        ):
            idn = constp.tile([128, 128], f32)
            nc.sync.dma_start(idn[:, :], ident[:, :])
            idn_b = constp.tile([128, 128], bf16)
            nc.vector.tensor_copy(idn_b[:, :], idn[:, :])
            wmain_sb = constp.tile([C, K2 * 64], bf16)
            nc.sync.dma_start(wmain_sb[:, :], wmain[:, :])
            wconv_sb = constp.tile([128, 6 * CH27], bf16)
            nc.sync.dma_start(wconv_sb[:, :], wconv[:, :])
            cg = constp.tile([128, NT * CHP], f32)
            nc.sync.dma_start(cg[:, :], cgrid[:, :])

            off_all = offp.tile([128, NT * CHP], f32)  # [p, t, ch32]
            out_sb = outp.tile([C, NPIX], f32)

            # ---------- Phase A: build xpd ----------
            with (
                tc.tile_pool(name="xs", bufs=1) as xsp,
                tc.tile_pool(name="nh", bufs=1) as nhp,
                tc.tile_pool(name="pst", bufs=2, space="PSUM") as pstp,
                tc.tile_pool(name="zs", bufs=1) as zsp,
            ):
                xs = xsp.tile([C, H * W], f32)
                nc.sync.dma_start(xs[:, :], xin[:, :])
                nh = nhp.tile([128, H * C], bf16)  # [w, h*64+c]
                for g in range(16):  # groups of 8 rows
                    ps = pstp.tile([128, 512], f32)
                    for j in range(8):
                        h = g * 8 + j
                        nc.tensor.transpose(
                            ps[:, j * 64:(j + 1) * 64],
                            xs[:, h * W:(h + 1) * W],
                            idn[:C, :C],
                        )
                    nc.scalar.copy(nh[:, g * 512:(g + 1) * 512], ps[:, :])

                # zero border strips of xpd
                zs = zsp.tile([128, 130], bf16)
                nc.vector.memset(zs[:, :], 0.0)
                # row r=0 both slots (slot1 rewritten by interior later)
                nc.sync.dma_start(
                    xpd[0:130 * 128].rearrange("(p x) -> p x", x=130), zs[:, :]
                )
                # row r=128 both slots (slot0 rewritten by interior later)
                nc.sync.dma_start(
                    xpd[128 * PW * 128:128 * PW * 128 + 130 * 128]
                    .rearrange("(p x) -> p x", x=130),
                    zs[:, :],
                )
                xv = xpd[0:PR * PW * 128].rearrange("(r y x) -> r y x", y=PW, x=128)
                nc.sync.dma_start(xv[0:65, 0, :], zs[0:65, 0:128])
                nc.sync.dma_start(xv[65:130, 0, :], zs[0:65, 0:128])
                nc.sync.dma_start(xv[0:65, PW - 1, :], zs[0:65, 0:128])
                nc.sync.dma_start(xv[65:130, PW - 1, :], zs[0:65, 0:128])
                # interior: slot0 at record-row h+1, slot1 at record-row h
                nh_v = nh[:, :].rearrange("p (h c) -> p h c", c=64)
                xpd_v = xpd[0:PR * PW * 128].rearrange(
                    "(r y s c) -> r y s c", y=PW, s=2, c=64)
                nc.sync.dma_start(
                    xpd_v[1:129, 1:129, 0, :].transpose([1, 0, 2]), nh_v
                )
                nc.sync.dma_start(
                    xpd_v[0:128, 1:129, 1, :].transpose([1, 0, 2]), nh_v
                )

            # ---------- Phase B: convs ----------
            with (
                tc.tile_pool(name="oms", bufs=1) as omsp,
                tc.tile_pool(name="cps", bufs=2, space="PSUM") as cpsp,
                tc.tile_pool(name="cos", bufs=2) as cosp,
                tc.tile_pool(name="pst2", bufs=2, space="PSUM") as pst2p,
            ):
                oms = omsp.tile([128, 66 * PW], bf16)
                nc.sync.dma_start(oms[0:C, :], om[:, :])
                nc.sync.dma_start(oms[C:128, 0:66 * PW - 1], om[:, 1:66 * PW])
                nc.vector.memset(oms[C:128, 66 * PW - 1:66 * PW], 0.0)
                mms = omsp.tile([128, 66 * PW], bf16)
                nc.sync.dma_start(mms[0:C, :], mm[:, :])
                nc.sync.dma_start(mms[C:128, 0:66 * PW - 1], mm[:, 1:66 * PW])
                nc.vector.memset(mms[C:128, 66 * PW - 1:66 * PW], 0.0)
                oms_v = oms[:, :].rearrange("c (r y) -> c r y", y=PW)
                mms_v = mms[:, :].rearrange("c (r y) -> c r y", y=PW)
                wconv_v = wconv_sb[:, :].rearrange("c (t j) -> c t j", j=CH27)

                for ch in range(NCHUNK):  # 4 output rows per chunk
                    r0 = ch * TCH
                    ps_o = cpsp.tile([2 * K2, 512], f32, tag="pso")
                    ps_m = cpsp.tile([K2, 512], f32, tag="psm")
                    for tg in range(6):
                        ki, kj = tg // 2, (tg % 2) * 2
                        rhs_o = oms_v[:, ki + r0: ki + r0 + TCH, kj: kj + W]
                        rhs_m = mms_v[:, ki + r0: ki + r0 + TCH, kj: kj + W]
                        nc.tensor.matmul(
                            ps_o[:, :], wconv_v[:, tg, 0:2 * K2], rhs_o,
                            start=(tg == 0), stop=(tg == 5),
                        )
                        nc.tensor.matmul(
                            ps_m[:, :], wconv_v[:, tg, 2 * K2:CH27], rhs_m,
                            start=(tg == 0), stop=(tg == 5),
                        )
                    co = cosp.tile([CHP, 512], f32, tag="co")
                    nc.vector.memset(co[:, :], 0.0)
                    nc.scalar.copy(co[0:2 * K2, :], ps_o[:, :])
                    nc.scalar.copy(co[32:32 + K2, :], ps_m[:, :])
                    ps_t = pst2p.tile([128, TCH * CHP], f32, tag="pst2")
                    for j in range(TCH):
                        nc.tensor.transpose(
                            ps_t[:, j * CHP:(j + 1) * CHP],
                            co[:, j * 128:(j + 1) * 128],
                            idn[:CHP, :CHP],
                        )
                    nc.scalar.copy(
                        off_all[:, (r0) * CHP:(r0 + TCH) * CHP], ps_t[:, :]
                    )

            # add base+bias grid -> off_all holds py, px, mc directly
            nc.vector.tensor_tensor(off_all[:, :], off_all[:, :], cg[:, :], alu.add)

            # ---------- Phase C: index & weight math ----------
            oav = off_all[:, :].rearrange("p (t j) -> p t j", j=CHP)
            py = oav[:, :, 0:2 * K2:2]
            px = oav[:, :, 1:2 * K2:2]
            mc = oav[:, :, 32:32 + K2]

            NK = NT * K2  # 576
            s_i32 = scalp.tile([128, NK], i32)
            y0 = scalp.tile([128, NK], f32)
            x0 = scalp.tile([128, NK], f32)
            tmp = scalp.tile([128, NK], f32)
            tmp2 = scalp.tile([128, NK], f32)
            wy = scalp.tile([128, NK], f32)
            wx = scalp.tile([128, NK], f32)
            ay0 = scalp.tile([128, NK], f32)
            ay1 = scalp.tile([128, NK], f32)
            bx0 = scalp.tile([128, NK], f32)
            bx1 = scalp.tile([128, NK], f32)
            msk2 = scalp.tile([128, NK], f32)
            idxf = scalp.tile([128, NK], f32)
            idx16 = scalp.tile([128, NK], i16)
            wq = wqp.tile([128, NT * K2 * 4], f32)  # [p, t, k, 4]
            wq2 = wqp.tile([128, NT * K2 * 4 * 2], bf16)  # pair-duplicated

            def floor_exact(dst, src):
                nc.vector.tensor_copy(s_i32[:, :], src)
                nc.vector.tensor_copy(dst, s_i32[:, :])
                nc.vector.tensor_tensor(tmp[:, :], dst, src, alu.is_gt)
                nc.vector.tensor_tensor(dst, dst, tmp[:, :], alu.subtract)

            floor_exact(y0[:, :], py)
            floor_exact(x0[:, :], px)
            nc.vector.tensor_tensor(wy[:, :], py, y0[:, :], alu.subtract)
            nc.vector.tensor_tensor(wx[:, :], px, x0[:, :], alu.subtract)

            # mask2 = 2*sigmoid(mc)
            nc.scalar.activation(msk2[:, :], mc, ACT.Sigmoid)
            nc.vector.tensor_scalar(msk2[:, :], msk2[:, :], 2.0, None, alu.mult)

            # validity
            def valid(dst, src, lo, hi):
                nc.vector.tensor_scalar(tmp[:, :], src, hi, None, alu.is_le)
                nc.vector.scalar_tensor_tensor(
                    dst, src, lo, tmp[:, :], alu.is_ge, alu.mult
                )

            valid(tmp2[:, :], y0[:, :], 0.0, 127.0)    # vy0
            # ay0 = (1-wy)*vy0*msk2
            nc.vector.tensor_scalar(ay0[:, :], wy[:, :], -1.0, 1.0, alu.mult, alu.add)
            nc.vector.tensor_tensor(ay0[:, :], ay0[:, :], tmp2[:, :], alu.mult)
            nc.vector.tensor_tensor(ay0[:, :], ay0[:, :], msk2[:, :], alu.mult)
            valid(tmp2[:, :], y0[:, :], -1.0, 126.0)   # vy1
            nc.vector.tensor_tensor(ay1[:, :], wy[:, :], tmp2[:, :], alu.mult)
            nc.vector.tensor_tensor(ay1[:, :], ay1[:, :], msk2[:, :], alu.mult)
            valid(tmp2[:, :], x0[:, :], 0.0, 127.0)    # vx0
            nc.vector.tensor_scalar(bx0[:, :], wx[:, :], -1.0, 1.0, alu.mult, alu.add)
            nc.vector.tensor_tensor(bx0[:, :], bx0[:, :], tmp2[:, :], alu.mult)
            valid(tmp2[:, :], x0[:, :], -1.0, 126.0)   # vx1
            nc.vector.tensor_tensor(bx1[:, :], wx[:, :], tmp2[:, :], alu.mult)

            wqv = wq[:, :].rearrange("p (t k c) -> p t k c", k=K2, c=4)
            # record order: [col0:(top,bot), col1:(top,bot)] = w00,w10,w01,w11
            nc.vector.tensor_tensor(wqv[:, :, :, 0], ay0[:, :].rearrange("p (t k) -> p t k", k=K2), bx0[:, :].rearrange("p (t k) -> p t k", k=K2), alu.mult)
            nc.vector.tensor_tensor(wqv[:, :, :, 1], ay1[:, :].rearrange("p (t k) -> p t k", k=K2), bx0[:, :].rearrange("p (t k) -> p t k", k=K2), alu.mult)
            nc.vector.tensor_tensor(wqv[:, :, :, 2], ay0[:, :].rearrange("p (t k) -> p t k", k=K2), bx1[:, :].rearrange("p (t k) -> p t k", k=K2), alu.mult)
            nc.vector.tensor_tensor(wqv[:, :, :, 3], ay1[:, :].rearrange("p (t k) -> p t k", k=K2), bx1[:, :].rearrange("p (t k) -> p t k", k=K2), alu.mult)

            # idx = (clip(y0,-1,127)+1)*130 + clip(x0,-1,127)+1
            nc.vector.tensor_scalar(tmp[:, :], y0[:, :], -1.0, 127.0, alu.max, alu.min)
            nc.vector.tensor_scalar(tmp2[:, :], x0[:, :], -1.0, 127.0, alu.max, alu.min)
            nc.vector.scalar_tensor_tensor(
                idxf[:, :], tmp[:, :], float(PW), tmp2[:, :], alu.mult, alu.add
            )
            nc.vector.tensor_scalar(idxf[:, :], idxf[:, :], float(PW + 1), None, alu.add)
            nc.vector.tensor_copy(idx16[:, :], idxf[:, :])

            # stage indices: didx[(t*9+k)*128 + p] = idx16[p, t, k]
            nc.sync.dma_start(
                didx[:].rearrange("(t k p) -> t k p", k=K2, p=128).transpose([2, 0, 1]),
                idx16[:, :].rearrange("p (t k) -> p t k", k=K2),
            )

            # ---------- Phase D/E ----------
            with (
                tc.tile_pool(name="idxa", bufs=1) as idxap,
                tc.tile_pool(name="rec", bufs=3) as recp,
                tc.tile_pool(name="samp", bufs=3) as sampp,
                tc.tile_pool(name="pst3", bufs=2, space="PSUM") as pst3p,
                tc.tile_pool(name="rhs", bufs=2) as rhsp,
                tc.tile_pool(name="ops", bufs=2, space="PSUM") as opsp,
            ):
                idxall = idxap.tile([128, NT * K2 * 128 // 16], i16)  # [128, 4608]
                didx_t = didx[:].rearrange("(s p) -> s p", p=16).transpose([1, 0])
                for r in range(8):
                    nc.sync.dma_start(idxall[r * 16:(r + 1) * 16, :], didx_t)

                wq2v = wq2[:, :].rearrange("p (n d) -> p n d", d=2)
                nc.vector.tensor_copy(wq2v[:, :, 0], wq[:, :])
                nc.vector.tensor_copy(wq2v[:, :, 1], wq[:, :])
                wmv = wmain_sb[:, :].rearrange("c (k o) -> c k o", o=64)
                xpd_src = xpd[:].rearrange("(r e) -> r e", e=128)[0:XPD_ROWS - 64, :]
                xpd_src = xpd_src.rearrange("r e -> r e")  # [rows,128]; elem 256 via 2 rows

                for chk in range(NCHUNK):
                    t0 = chk * TCH
                    rec = recp.tile([128, TCH * K2 * REC], bf16, tag="rec")
                    xpd_gsrc = AP(
                        xpd[:].tensor, 0,
                        [[128, XPD_ROWS - 2], [1, REC]],
                    )
                    nc.gpsimd.dma_gather(
                        rec[:, :].rearrange("p (n e) -> p n e", e=REC),
                        xpd_gsrc,
                        idxall[:, chk * (IDX_PER_CHUNK // 16):(chk + 1) * (IDX_PER_CHUNK // 16)],
                        num_idxs=IDX_PER_CHUNK,
                        num_idxs_reg=IDX_PER_CHUNK,
                        elem_size=REC,
                        elem_step=128,
                        single_packet=False,
                    )
                    # scale by corner weights (in-place), combine corners.
                    # weights operand reads step-1 duplicated pairs so the DVE
                    # 2x bf16 packing mode stays eligible (step-0 broadcast
                    # would force 1x).
                    recv = rec[:, :].rearrange("p (n c e) -> p n c e", c=4, e=64)
                    recv2 = rec[:, :].rearrange(
                        "p (m e2 d) -> p m e2 d", e2=32, d=2
                    )  # m = (n,c) folded: 144 per chunk
                    wbase = wq2[:, t0 * K2 * 4 * 2:(t0 + TCH) * K2 * 4 * 2]
                    wap = AP(
                        wbase.tensor, wbase.offset,
                        [list(wbase.ap[0]), [2, TCH * K2 * 4], [0, 32], [1, 2]],
                    )
                    nc.vector.tensor_tensor(recv2, recv2, wap, alu.mult)
                    samp = sampp.tile([128, TCH * K2 * 64], bf16, tag="samp")
                    sampv = samp[:, :].rearrange("p (n e) -> p n e", e=64)
                    nc.vector.tensor_tensor(
                        sampv,
                        recv[:, :, 0, :], recv[:, :, 1, :], alu.add,
                    )
                    nc.vector.tensor_tensor(
                        sampv, sampv, recv[:, :, 2, :], alu.add,
                    )
                    nc.vector.tensor_tensor(
                        sampv, sampv, recv[:, :, 3, :], alu.add,
                    )
                    ops_t = opsp.tile([C, 512], f32, tag="ops")
                    for kg in range(3):
                        ps3 = pst3p.tile([C, 3 * 512], bf16, tag="ps3")
                        for k3 in range(3):
                            k = kg * 3 + k3
                            for j in range(TCH):
                                nc.tensor.transpose(
                                    ps3[:, k3 * 512 + j * 128:k3 * 512 + (j + 1) * 128],
                                    sampv[:, (j * K2 + k), :],
                                    idn_b[:, :],
                                )
                        rhs = rhsp.tile([C, 3 * 512], bf16, tag="rhs")
                        nc.scalar.copy(rhs[:, :], ps3[:, :])
                        for k3 in range(3):
                            k = kg * 3 + k3
                            nc.tensor.matmul(
                                ops_t[:, :], wmv[:, k, :],
                                rhs[:, k3 * 512:(k3 + 1) * 512],
                                start=(k == 0), stop=(k == K2 - 1),
                            )
                    nc.scalar.copy(out_sb[:, t0 * W:(t0 + TCH) * W], ops_t[:, :])

            nc.sync.dma_start(out[:, :], out_sb[:, :])

    nc.compile()
    return nc


def _prep_core_inputs(inputs, core):
    x = np.asarray(inputs["x"], np.float32)
    omap = np.asarray(inputs["offset_map"], np.float32)
    mmap = np.asarray(inputs["modulator_map"], np.float32)
    ow = np.asarray(inputs["offset_w"], np.float32)
    ob = np.asarray(inputs["offset_b"], np.float32)
    mw = np.asarray(inputs["mod_w"], np.float32)
    mb = np.asarray(inputs["mod_b"], np.float32)
    wt = np.asarray(inputs["weight"], np.float32)
    import ml_dtypes

    b, hh = core // 2, core % 2
    h0 = hh * HH

    def slab(a):
        s = np.zeros((C, 66, PW), np.float32)
        lo, hi = h0 - 1, h0 + HH + 1  # global rows [lo, hi)
        glo, ghi = max(lo, 0), min(hi, H)
        s[:, glo - lo: ghi - lo, 1:W + 1] = a[b, :, glo:ghi, :]
        return s.reshape(C, 66 * PW).astype(ml_dtypes.bfloat16)

    wmain = np.ascontiguousarray(
        wt.reshape(C, C, K2).transpose(1, 2, 0).reshape(C, K2 * 64)
    ).astype(ml_dtypes.bfloat16)
    catw = np.concatenate([ow.reshape(2 * K2, C, K2), mw.reshape(K2, C, K2)], 0)
    # [128, 6, 27]: tap-group tg=ki*2+{0:pair kj=0|1, 1:single kj=2 (top 0)}
    wconv = np.zeros((128, 6, CH27), np.float32)
    for ki in range(K):
        wconv[0:C, ki * 2 + 0, :] = catw[:, :, ki * K + 0].T
        wconv[C:128, ki * 2 + 0, :] = catw[:, :, ki * K + 1].T
        wconv[0:C, ki * 2 + 1, :] = catw[:, :, ki * K + 2].T
    wconv = np.ascontiguousarray(wconv.reshape(128, 6 * CH27)).astype(
        ml_dtypes.bfloat16
    )

    cgrid = np.zeros((128, NT, CHP), np.float32)
    p = np.arange(128)
    t = np.arange(NT)
    for k in range(K2):
        ki, kj = k // K, k % K
        cgrid[:, :, 2 * k] = ob[2 * k] + (h0 + t[None, :]) + ki - 1
        cgrid[:, :, 2 * k + 1] = ob[2 * k + 1] + p[:, None] + kj - 1
        cgrid[:, :, 32 + k] = mb[k]
    ident = np.eye(128, dtype=np.float32)

    return {
        "xin": x[b].reshape(C, H * W).copy(),
        "om": slab(omap),
        "mm": slab(mmap),
        "wmain": wmain,
        "wconv": wconv,
        "cgrid": cgrid.reshape(128, NT * CHP).copy(),
        "ident": ident,
    }


def get_module():
    import os
    phase = int(os.environ.get("KPHASE", "4"))
    key = ("nc", phase)
    if key not in _CACHE:
        _CACHE[key] = _build_module(phase)
    return _CACHE[key]


def kernel(**inputs) -> np.ndarray:
    import os
    from concourse.bass_utils import run_bass_kernel_spmd

    nc = get_module()
    in_maps = [_prep_core_inputs(inputs, c) for c in range(NCO)]
    trace = bool(int(os.environ.get("KBENCH_TRACE", "0")))
    res = run_bass_kernel_spmd(nc, in_maps, core_ids=list(range(NCO)), trace=trace)
    _CACHE["last_results"] = res
    out = np.zeros((B, C, H, W), np.float32)
    for c in range(NCO):
        b, hh = c // 2, c % 2
        out[b, :, hh * HH:(hh + 1) * HH, :] = res.results[c]["out"].reshape(C, HH, W)
    return out


# revision 28
# speedup vs baseline: 1.3062x; 1.0835x over previous
"""DeformableConv2d Trainium2 kernel.

Shapes (hardcoded): x[4,64,128,128], offset/modulator maps [4,64,128,128],
offset_w[18,64,3,3], offset_b[18], mod_w[9,64,3,3], mod_b[9], weight[64,64,3,3].
Output [4,64,128,128] f32.

Sharding: 8 cores; core c -> batch b=c//2, row-half hh=c%2 (output rows
h0=64*hh .. h0+64). Each core keeps the full image of its batch for the
(clipped) bilinear gathers.

Algorithm per core:
  A) Build padded NHWC "pair" image xpd[r, cw, 2, 64] in DRAM from x (PE
     transposes). Record at (r,cw) = pixels (r-1,cw-1),(r-1,cw),(r,cw-1),(r,cw)
     of the image with 1-px zero border; one 1KB dma_gather record = all 4
     bilinear corners for all 64 channels.
  B) offset/mod convs as 9-tap accumulated matmuls (bf16 in, f32 psum),
     PE-transpose to pixel-on-partition layout, add host-baked base+bias grid.
  C) Exact floor/clip/validity/bilinear-weight math on DVE (f32), int16
     gather indices staged via DRAM into the 16-partition wrapped layout.
  D) dma_gather 2x2x64 records; scale by per-corner weights (broadcast AP),
     combine corners, PE-transpose, accumulate out = sum_k W_k^T @ samp_k.
"""

import numpy as np

B, C, H, W = 4, 64, 128, 128
K = 3
K2 = 9
NCO = 8          # cores
HH = H // 2      # rows per core (64)
NPIX = HH * W    # 8192 pixels per core
NT = HH          # 64 pixel tiles of 128 (one image row each)
PW = W + 2       # 130 padded width
PR = H + 2       # 130 padded rows
REC = 2 * 2 * C  # 256 f32 per gather record
XPD_ROWS = (PR - 1) * PW + PW + 64   # generous slack beyond max idx reads
CH27 = 2 * K2 + K2                   # 18 offset + 9 mod channels
CHP = 48         # padded channel layout: 0:18 offs, 32:41 mod, rest junk
TCH = 4          # pixel tiles per matmul chunk (512 px)
NCHUNK = NT // TCH                   # 16 chunks
IDX_PER_CHUNK = TCH * K2 * 128       # 4608

_CACHE = {}


def _build_module(phase=4):
    import concourse.bass as bass
    import concourse.bacc as bacc
    import concourse.mybir as mybir
    from concourse.tile import TileContext
    from concourse.alu_op_type import AluOpType as alu
    from concourse.bass_types import AP
    from concourse import library_config

    f32 = mybir.dt.float32
    bf16 = mybir.dt.bfloat16
    i16 = mybir.dt.int16
    i32 = mybir.dt.int32
    ACT = mybir.ActivationFunctionType

    nc = bacc.Bacc()

    xin = nc.dram_tensor("xin", (C, H * W), f32, kind="ExternalInput")
    om = nc.dram_tensor("om", (C, 66 * PW), bf16, kind="ExternalInput")
    mm = nc.dram_tensor("mm", (C, 66 * PW), bf16, kind="ExternalInput")
    wmain = nc.dram_tensor("wmain", (C, K2 * 64), bf16, kind="ExternalInput")
    wconv = nc.dram_tensor("wconv", (128, 6 * CH27), bf16, kind="ExternalInput")
    cgrid = nc.dram_tensor("cgrid", (128, NT * CHP), f32, kind="ExternalInput")
    ident = nc.dram_tensor("ident", (128, 128), f32, kind="ExternalInput")
    out = nc.dram_tensor("out", (C, NPIX), f32, kind="ExternalOutput")

    xpd = nc.dram_tensor("xpd", (XPD_ROWS * 128,), bf16, kind="Internal")
    didx = nc.dram_tensor("didx", (NT * K2 * 128,), i16, kind="Internal")

    with TileContext(nc) as tc:
        # ---------------- persistent pools ----------------
        with (
            tc.tile_pool(name="const", bufs=1) as constp,
            tc.tile_pool(name="offall", bufs=1) as offp,
            tc.tile_pool(name="scal", bufs=1) as scalp,
            tc.tile_pool(name="wq", bufs=1) as wqp,
            tc.tile_pool(name="outsb", bufs=1) as outp,
        ):
            idn = constp.tile([128, 128], f32)
            nc.sync.dma_start(idn[:, :], ident[:, :])
            idn_b = constp.tile([128, 128], bf16)
            nc.vector.tensor_copy(idn_b[:, :], idn[:, :])
            wmain_sb = constp.tile([C, K2 * 64], bf16)
            nc.sync.dma_start(wmain_sb[:, :], wmain[:, :])
            wconv_sb = constp.tile([128, 6 * CH27], bf16)
            nc.sync.dma_start(wconv_sb[:, :], wconv[:, :])
            cg = constp.tile([128, NT * CHP], f32)
            nc.sync.dma_start(cg[:, :], cgrid[:, :])

            off_all = offp.tile([128, NT * CHP], f32)  # [p, t, ch32]
            out_sb = outp.tile([C, NPIX], f32)

            # ---------- Phase A: build xpd ----------
            with (
                tc.tile_pool(name="xs", bufs=1) as xsp,
                tc.tile_pool(name="nh", bufs=1) as nhp,
                tc.tile_pool(name="pst", bufs=2, space="PSUM") as pstp,
                tc.tile_pool(name="zs", bufs=1) as zsp,
            ):
                xsb = xsp.tile([C, H * W], bf16)
                nc.gpsimd.dma_start(xsb[:, :], xin[:, :])  # SWDGE f32->bf16 cast
                nh = nhp.tile([128, H * C], bf16)  # [w, h*64+c]
                for g in range(16):  # groups of 8 rows
                    ps = pstp.tile([128, 512], bf16)
                    for j in range(8):
                        h = g * 8 + j
                        nc.tensor.transpose(
                            ps[:, j * 64:(j + 1) * 64],
                            xsb[:, h * W:(h + 1) * W],
                            idn_b[:C, :C],
                        )
                    nc.scalar.copy(nh[:, g * 512:(g + 1) * 512], ps[:, :])

                # zero border strips of xpd
                zs = zsp.tile([128, 130], bf16)
                nc.vector.memset(zs[:, :], 0.0)
                # row r=0 both slots (slot1 rewritten by interior later)
                nc.sync.dma_start(
                    xpd[0:130 * 128].rearrange("(p x) -> p x", x=130), zs[:, :]
                )
                # row r=128 both slots (slot0 rewritten by interior later)
                nc.sync.dma_start(
                    xpd[128 * PW * 128:128 * PW * 128 + 130 * 128]
                    .rearrange("(p x) -> p x", x=130),
                    zs[:, :],
                )
                xv = xpd[0:PR * PW * 128].rearrange("(r y x) -> r y x", y=PW, x=128)
                nc.sync.dma_start(xv[0:65, 0, :], zs[0:65, 0:128])
                nc.sync.dma_start(xv[65:130, 0, :], zs[0:65, 0:128])
                nc.sync.dma_start(xv[0:65, PW - 1, :], zs[0:65, 0:128])
                nc.sync.dma_start(xv[65:130, PW - 1, :], zs[0:65, 0:128])
                # interior. record-row r holds (slot0=padded row r, slot1=
                # padded row r+1) = nh rows r-1, r: one DMA with contiguous
                # 256B (s,c) runs covers r=1..127 both slots; edge slots done
                # separately. (slot0 r=0 and slot1 r=128 stay zero.)
                nh_v = nh[:, :].rearrange("p (h c) -> p h c", c=64)
                xpd_v = xpd[0:PR * PW * 128].rearrange(
                    "(r y s c) -> r y s c", y=PW, s=2, c=64)
                nh_pair = AP(
                    nh[:, :].tensor, nh[:, :].offset,
                    [list(nh[:, :].ap[0]), [64, 127], [1, 128]],
                )
                nc.sync.dma_start(
                    xpd_v[1:128, 1:129, :, :]
                    .transpose([1, 0, 2, 3])
                    .rearrange("y r s c -> y r (s c)"),
                    nh_pair,
                )
                nc.sync.dma_start(xpd_v[0, 1:129, 1, :], nh_v[:, 0, :])
                nc.sync.dma_start(xpd_v[128, 1:129, 0, :], nh_v[:, 127, :])

            # ---------- Phase B: convs ----------
            with (
                tc.tile_pool(name="oms", bufs=1) as omsp,
                tc.tile_pool(name="cps", bufs=2, space="PSUM") as cpsp,
                tc.tile_pool(name="cos", bufs=2) as cosp,
                tc.tile_pool(name="pst2", bufs=2, space="PSUM") as pst2p,
            ):
                oms = omsp.tile([128, 66 * PW], bf16)
                nc.sync.dma_start(oms[0:C, :], om[:, :])
                nc.sync.dma_start(oms[C:128, 0:66 * PW - 1], om[:, 1:66 * PW])
                nc.vector.memset(oms[C:128, 66 * PW - 1:66 * PW], 0.0)
                mms = omsp.tile([128, 66 * PW], bf16)
                nc.sync.dma_start(mms[0:C, :], mm[:, :])
                nc.sync.dma_start(mms[C:128, 0:66 * PW - 1], mm[:, 1:66 * PW])
                nc.vector.memset(mms[C:128, 66 * PW - 1:66 * PW], 0.0)
                oms_v = oms[:, :].rearrange("c (r y) -> c r y", y=PW)
                mms_v = mms[:, :].rearrange("c (r y) -> c r y", y=PW)
                wconv_v = wconv_sb[:, :].rearrange("c (t j) -> c t j", j=CH27)

                for ch in range(NCHUNK):  # 4 output rows per chunk
                    r0 = ch * TCH
                    ps_o = cpsp.tile([2 * K2, 512], f32, tag="pso")
                    ps_m = cpsp.tile([K2, 512], f32, tag="psm")
                    for tg in range(6):
                        ki, kj = tg // 2, (tg % 2) * 2
                        rhs_o = oms_v[:, ki + r0: ki + r0 + TCH, kj: kj + W]
                        rhs_m = mms_v[:, ki + r0: ki + r0 + TCH, kj: kj + W]
                        nc.tensor.matmul(
                            ps_o[:, :], wconv_v[:, tg, 0:2 * K2], rhs_o,
                            start=(tg == 0), stop=(tg == 5),
                        )
                        nc.tensor.matmul(
                            ps_m[:, :], wconv_v[:, tg, 2 * K2:CH27], rhs_m,
                            start=(tg == 0), stop=(tg == 5),
                        )
                    co = cosp.tile([CHP, 512], f32, tag="co")
                    nc.vector.memset(co[:, :], 0.0)
                    nc.scalar.copy(co[0:2 * K2, :], ps_o[:, :])
                    nc.scalar.copy(co[32:32 + K2, :], ps_m[:, :])
                    ps_t = pst2p.tile([128, TCH * CHP], f32, tag="pst2")
                    for j in range(TCH):
                        nc.tensor.transpose(
                            ps_t[:, j * CHP:(j + 1) * CHP],
                            co[:, j * 128:(j + 1) * 128],
                            idn[:CHP, :CHP],
                        )
                    nc.scalar.copy(
                        off_all[:, (r0) * CHP:(r0 + TCH) * CHP], ps_t[:, :]
                    )

            # add base+bias grid -> off_all holds py, px, mc directly
            nc.vector.tensor_tensor(off_all[:, :], off_all[:, :], cg[:, :], alu.add)

            # ---------- Phase C: index & weight math ----------
            oav = off_all[:, :].rearrange("p (t j) -> p t j", j=CHP)
            py = oav[:, :, 0:2 * K2:2]
            px = oav[:, :, 1:2 * K2:2]
            mc = oav[:, :, 32:32 + K2]

            NK = NT * K2  # 576
            s_i32 = scalp.tile([128, NK], i32)
            y0 = scalp.tile([128, NK], f32)
            x0 = scalp.tile([128, NK], f32)
            tmp = scalp.tile([128, NK], f32)
            tmp2 = scalp.tile([128, NK], f32)
            wy = scalp.tile([128, NK], f32)
            wx = scalp.tile([128, NK], f32)
            ay0 = scalp.tile([128, NK], f32)
            ay1 = scalp.tile([128, NK], f32)
            bx0 = scalp.tile([128, NK], f32)
            bx1 = scalp.tile([128, NK], f32)
            msk2 = scalp.tile([128, NK], f32)
            idxf = scalp.tile([128, NK], f32)
            idx16 = scalp.tile([128, NK], i16)
            wq = wqp.tile([128, NT * K2 * 4], f32)  # [p, t, k, 4]
            wq2 = wqp.tile([128, NT * K2 * 4 * 2], bf16)  # pair-duplicated

            def floor_exact(dst, src):
                nc.vector.tensor_copy(s_i32[:, :], src)
                nc.vector.tensor_copy(dst, s_i32[:, :])
                nc.vector.tensor_tensor(tmp[:, :], dst, src, alu.is_gt)
                nc.vector.tensor_tensor(dst, dst, tmp[:, :], alu.subtract)

            floor_exact(y0[:, :], py)
            floor_exact(x0[:, :], px)
            nc.vector.tensor_tensor(wy[:, :], py, y0[:, :], alu.subtract)
            nc.vector.tensor_tensor(wx[:, :], px, x0[:, :], alu.subtract)

            # mask2 = 2*sigmoid(mc)
            nc.scalar.activation(msk2[:, :], mc, ACT.Sigmoid)
            nc.vector.tensor_scalar(msk2[:, :], msk2[:, :], 2.0, None, alu.mult)

            # validity
            def valid(dst, src, lo, hi):
                nc.vector.tensor_scalar(tmp[:, :], src, hi, None, alu.is_le)
                nc.vector.scalar_tensor_tensor(
                    dst, src, lo, tmp[:, :], alu.is_ge, alu.mult
                )

            valid(tmp2[:, :], y0[:, :], 0.0, 127.0)    # vy0
            # ay0 = (1-wy)*vy0*msk2
            nc.vector.tensor_scalar(ay0[:, :], wy[:, :], -1.0, 1.0, alu.mult, alu.add)
            nc.vector.tensor_tensor(ay0[:, :], ay0[:, :], tmp2[:, :], alu.mult)
            nc.vector.tensor_tensor(ay0[:, :], ay0[:, :], msk2[:, :], alu.mult)
            valid(tmp2[:, :], y0[:, :], -1.0, 126.0)   # vy1
            nc.vector.tensor_tensor(ay1[:, :], wy[:, :], tmp2[:, :], alu.mult)
            nc.vector.tensor_tensor(ay1[:, :], ay1[:, :], msk2[:, :], alu.mult)
            valid(tmp2[:, :], x0[:, :], 0.0, 127.0)    # vx0
            nc.vector.tensor_scalar(bx0[:, :], wx[:, :], -1.0, 1.0, alu.mult, alu.add)
            nc.vector.tensor_tensor(bx0[:, :], bx0[:, :], tmp2[:, :], alu.mult)
            valid(tmp2[:, :], x0[:, :], -1.0, 126.0)   # vx1
            nc.vector.tensor_tensor(bx1[:, :], wx[:, :], tmp2[:, :], alu.mult)

            wqv = wq[:, :].rearrange("p (t k c) -> p t k c", k=K2, c=4)
            # record order: [col0:(top,bot), col1:(top,bot)] = w00,w10,w01,w11
            nc.vector.tensor_tensor(wqv[:, :, :, 0], ay0[:, :].rearrange("p (t k) -> p t k", k=K2), bx0[:, :].rearrange("p (t k) -> p t k", k=K2), alu.mult)
            nc.vector.tensor_tensor(wqv[:, :, :, 1], ay1[:, :].rearrange("p (t k) -> p t k", k=K2), bx0[:, :].rearrange("p (t k) -> p t k", k=K2), alu.mult)
            nc.vector.tensor_tensor(wqv[:, :, :, 2], ay0[:, :].rearrange("p (t k) -> p t k", k=K2), bx1[:, :].rearrange("p (t k) -> p t k", k=K2), alu.mult)
            nc.vector.tensor_tensor(wqv[:, :, :, 3], ay1[:, :].rearrange("p (t k) -> p t k", k=K2), bx1[:, :].rearrange("p (t k) -> p t k", k=K2), alu.mult)

            # idx = (clip(y0,-1,127)+1)*130 + clip(x0,-1,127)+1
            nc.vector.tensor_scalar(tmp[:, :], y0[:, :], -1.0, 127.0, alu.max, alu.min)
            nc.vector.tensor_scalar(tmp2[:, :], x0[:, :], -1.0, 127.0, alu.max, alu.min)
            nc.vector.scalar_tensor_tensor(
                idxf[:, :], tmp[:, :], float(PW), tmp2[:, :], alu.mult, alu.add
            )
            nc.vector.tensor_scalar(idxf[:, :], idxf[:, :], float(PW + 1), None, alu.add)
            nc.vector.tensor_copy(idx16[:, :], idxf[:, :])

            # stage indices: didx[(t*9+k)*128 + p] = idx16[p, t, k]
            nc.sync.dma_start(
                didx[:].rearrange("(t k p) -> t k p", k=K2, p=128).transpose([2, 0, 1]),
                idx16[:, :].rearrange("p (t k) -> p t k", k=K2),
            )

            # ---------- Phase D/E ----------
            with (
                tc.tile_pool(name="idxa", bufs=1) as idxap,
                tc.tile_pool(name="rec", bufs=3) as recp,
                tc.tile_pool(name="samp", bufs=3) as sampp,
                tc.tile_pool(name="pst3", bufs=3, space="PSUM") as pst3p,
                tc.tile_pool(name="rhs", bufs=3) as rhsp,
                tc.tile_pool(name="ops", bufs=2, space="PSUM") as opsp,
            ):
                idxall = idxap.tile([128, NT * K2 * 128 // 16], i16)  # [128, 4608]
                didx_t = didx[:].rearrange("(s p) -> s p", p=16).transpose([1, 0])
                for r in range(8):
                    nc.sync.dma_start(idxall[r * 16:(r + 1) * 16, :], didx_t)

                wq2v = wq2[:, :].rearrange("p (n d) -> p n d", d=2)
                nc.vector.tensor_copy(wq2v[:, :, 0], wq[:, :])
                nc.vector.tensor_copy(wq2v[:, :, 1], wq[:, :])
                wmv = wmain_sb[:, :].rearrange("c (k o) -> c k o", o=64)
                xpd_src = xpd[:].rearrange("(r e) -> r e", e=128)[0:XPD_ROWS - 64, :]
                xpd_src = xpd_src.rearrange("r e -> r e")  # [rows,128]; elem 256 via 2 rows

                for chk in range(NCHUNK):
                    t0 = chk * TCH
                    rec = recp.tile([128, TCH * K2 * REC], bf16, tag="rec")
                    xpd_gsrc = AP(
                        xpd[:].tensor, 0,
                        [[128, XPD_ROWS - 2], [1, REC]],
                    )
                    nc.gpsimd.dma_gather(
                        rec[:, :].rearrange("p (n e) -> p n e", e=REC),
                        xpd_gsrc,
                        idxall[:, chk * (IDX_PER_CHUNK // 16):(chk + 1) * (IDX_PER_CHUNK // 16)],
                        num_idxs=IDX_PER_CHUNK,
                        num_idxs_reg=IDX_PER_CHUNK,
                        elem_size=REC,
                        elem_step=128,
                        single_packet=False,
                    )
                    # scale by corner weights (in-place), combine corners.
                    # weights operand reads step-1 duplicated pairs so the DVE
                    # 2x bf16 packing mode stays eligible (step-0 broadcast
                    # would force 1x).
                    recv = rec[:, :].rearrange("p (n c e) -> p n c e", c=4, e=64)
                    recv2 = rec[:, :].rearrange(
                        "p (m e2 d) -> p m e2 d", e2=32, d=2
                    )  # m = (n,c) folded: 144 per chunk
                    wbase = wq2[:, t0 * K2 * 4 * 2:(t0 + TCH) * K2 * 4 * 2]
                    wap = AP(
                        wbase.tensor, wbase.offset,
                        [list(wbase.ap[0]), [2, TCH * K2 * 4], [0, 32], [1, 2]],
                    )
                    nc.vector.tensor_tensor(recv2, recv2, wap, alu.mult)
                    samp = sampp.tile([128, TCH * K2 * 64], bf16, tag="samp")
                    sampv = samp[:, :].rearrange("p (n e) -> p n e", e=64)
                    nc.vector.tensor_tensor(
                        sampv,
                        recv[:, :, 0, :], recv[:, :, 1, :], alu.add,
                    )
                    nc.vector.tensor_tensor(
                        sampv, sampv, recv[:, :, 2, :], alu.add,
                    )
                    nc.vector.tensor_tensor(
                        sampv, sampv, recv[:, :, 3, :], alu.add,
                    )
                    ops_t = opsp.tile([C, 512], f32, tag="ops")
                    for kg in range(3):
                        ps3 = pst3p.tile([C, 3 * 512], bf16, tag="ps3")
                        for k3 in range(3):
                            k = kg * 3 + k3
                            for j in range(TCH):
                                nc.tensor.transpose(
                                    ps3[:, k3 * 512 + j * 128:k3 * 512 + (j + 1) * 128],
                                    sampv[:, (j * K2 + k), :],
                                    idn_b[:, :],
                                )
                        rhs = rhsp.tile([C, 3 * 512], bf16, tag="rhs")
                        nc.scalar.copy(rhs[:, :], ps3[:, :])
                        for k3 in range(3):
                            k = kg * 3 + k3
                            nc.tensor.matmul(
                                ops_t[:, :], wmv[:, k, :],
                                rhs[:, k3 * 512:(k3 + 1) * 512],
                                start=(k == 0), stop=(k == K2 - 1),
                            )
                    nc.scalar.copy(out_sb[:, t0 * W:(t0 + TCH) * W], ops_t[:, :])
                    nc.sync.dma_start(
                        out[:, t0 * W:(t0 + TCH) * W],
                        out_sb[:, t0 * W:(t0 + TCH) * W],
                    )

    nc.compile()
    return nc


def _prep_core_inputs(inputs, core):
    x = np.asarray(inputs["x"], np.float32)
    omap = np.asarray(inputs["offset_map"], np.float32)
    mmap = np.asarray(inputs["modulator_map"], np.float32)
    ow = np.asarray(inputs["offset_w"], np.float32)
    ob = np.asarray(inputs["offset_b"], np.float32)
    mw = np.asarray(inputs["mod_w"], np.float32)
    mb = np.asarray(inputs["mod_b"], np.float32)
    wt = np.asarray(inputs["weight"], np.float32)
    import ml_dtypes

    b, hh = core // 2, core % 2
    h0 = hh * HH

    def slab(a):
        s = np.zeros((C, 66, PW), np.float32)
        lo, hi = h0 - 1, h0 + HH + 1  # global rows [lo, hi)
        glo, ghi = max(lo, 0), min(hi, H)
        s[:, glo - lo: ghi - lo, 1:W + 1] = a[b, :, glo:ghi, :]
        return s.reshape(C, 66 * PW).astype(ml_dtypes.bfloat16)

    wmain = np.ascontiguousarray(
        wt.reshape(C, C, K2).transpose(1, 2, 0).reshape(C, K2 * 64)
    ).astype(ml_dtypes.bfloat16)
    catw = np.concatenate([ow.reshape(2 * K2, C, K2), mw.reshape(K2, C, K2)], 0)
    # [128, 6, 27]: tap-group tg=ki*2+{0:pair kj=0|1, 1:single kj=2 (top 0)}
    wconv = np.zeros((128, 6, CH27), np.float32)
    for ki in range(K):
        wconv[0:C, ki * 2 + 0, :] = catw[:, :, ki * K + 0].T
        wconv[C:128, ki * 2 + 0, :] = catw[:, :, ki * K + 1].T
        wconv[0:C, ki * 2 + 1, :] = catw[:, :, ki * K + 2].T
    wconv = np.ascontiguousarray(wconv.reshape(128, 6 * CH27)).astype(
        ml_dtypes.bfloat16
    )

    cgrid = np.zeros((128, NT, CHP), np.float32)
    p = np.arange(128)
    t = np.arange(NT)
    for k in range(K2):
        ki, kj = k // K, k % K
        cgrid[:, :, 2 * k] = ob[2 * k] + (h0 + t[None, :]) + ki - 1
        cgrid[:, :, 2 * k + 1] = ob[2 * k + 1] + p[:, None] + kj - 1
        cgrid[:, :, 32 + k] = mb[k]
    ident = np.eye(128, dtype=np.float32)

    return {
        "xin": x[b].reshape(C, H * W).copy(),
        "om": slab(omap),
        "mm": slab(mmap),
        "wmain": wmain,
        "wconv": wconv,
        "cgrid": cgrid.reshape(128, NT * CHP).copy(),
        "ident": ident,
    }


def get_module():
    import os
    phase = int(os.environ.get("KPHASE", "4"))
    key = ("nc", phase)
    if key not in _CACHE:
        _CACHE[key] = _build_module(phase)
    return _CACHE[key]


def kernel(**inputs) -> np.ndarray:
    import os
    from concourse.bass_utils import run_bass_kernel_spmd

    nc = get_module()
    in_maps = [_prep_core_inputs(inputs, c) for c in range(NCO)]
    trace = bool(int(os.environ.get("KBENCH_TRACE", "0")))
    res = run_bass_kernel_spmd(nc, in_maps, core_ids=list(range(NCO)), trace=trace)
    _CACHE["last_results"] = res
    out = np.zeros((B, C, H, W), np.float32)
    for c in range(NCO):
        b, hh = c // 2, c % 2
        out[b, :, hh * HH:(hh + 1) * HH, :] = res.results[c]["out"].reshape(C, HH, W)
    return out
